# revision 25
# baseline (speedup 1.0000x reference)
"""BitLinear fake-quant GEMM on 8 trn2 NeuronCores, data-parallel over batch.

Per core: y[s,o] = round(clip(x/a_scale*127)) @ clip(round(w/w_scale),-1,1)^T
          * (w_scale * a_scale / 127),  a_scale = rowmax|x| + eps.

Quantized activations are integers |v|<=127 and weights are ternary, so a
bf16 matmul with fp32 PSUM accumulation is exact integer arithmetic.

Pipelined around the PE floor (512 N=512 matmuls ~111us @2.4GHz).
Tile's cross-engine sync uses per-engine counting semaphores plus
sem-forwarding barrier instructions that land in-order in consumer queues;
every extra producer engine adds barriers that can block an unrelated
consumer.  So the pipeline is one engine per stage, one handoff per stage:
  GpSimd queue: x loads (+ weights, behind x0) | DVE: reduce, 127/a, quant |
  Sync queue: transposes | PE: matmuls | Scalar: epi stat, PSUM epilogue,
  y stores
- weight ternarization on host (bit-identical f32 round-half-even mirror of
  the on-device chain it replaces); 2 MB bf16 ternary weight rides the x
  queue behind x pair 0 so the first quant chain owns the DMA engines
- stats/quant run two pairs ahead of the matmuls; all 16 x loads are
  issued up front on their own queue, self-paced by xg-pool recycling
- x loaded and y stored as bf16 (tolerance 2e-2; bf16 is ~2^-9)
"""

import os
import sys

import numpy as np

sys.path.insert(0, "/opt/trn_rl_repo")

import concourse.bacc as bacc
import concourse.mybir as mybir
import concourse.tile as tile
from concourse.bass_utils import run_bass_kernel_spmd

F32 = mybir.dt.float32
BF16 = mybir.dt.bfloat16
AF = mybir.ActivationFunctionType
ALU = mybir.AluOpType

B = 8      # batches == cores
S = 4096   # rows per core
D = 1024   # in features (contraction)
O = 1024   # out features
P = 128
KB = D // P
NT = S // P          # 32 s-tiles
NP = NT // 2         # 16 s-tile pairs
RND = 12582912.0     # 1.5*2**23: (z+RND)-RND == round-half-even(z) for |z|<2**22
EPS = 1e-8

_CACHE = {}
TRACE_DIR = None


def _build():
    nc = bacc.Bacc("TRN2", target_bir_lowering=False, debug=False)
    x_d = nc.dram_tensor("x", [S, D], BF16, kind="ExternalInput")
    w_d = nc.dram_tensor("wqT", [D, O], BF16, kind="ExternalInput")
    wsc_d = nc.dram_tensor("wsc", [P, 1], F32, kind="ExternalInput")
    y_d = nc.dram_tensor("y", [S, O], BF16, kind="ExternalOutput")
    xa, wa, sca, ya = x_d.ap(), w_d.ap(), wsc_d.ap(), y_d.ap()

    xa3 = xa.rearrange("(a p) d -> p a d", p=P)   # [P, NT, D]
    ya3 = ya.rearrange("(a p) o -> p a o", p=P)   # [P, NT, O]
    wa3 = wa.rearrange("(a p) o -> p a o", p=P)   # [P, KB, O]

    with tile.TileContext(nc) as tc:
        with (
            tc.tile_pool(name="wqT", bufs=1) as wqT_p,
            tc.tile_pool(name="xg", bufs=12) as xg_p,
            tc.tile_pool(name="stat", bufs=10) as stat_p,
            tc.tile_pool(name="tq", bufs=4) as tq_p,
            tc.tile_pool(name="aq2", bufs=8) as aq2_p,
            tc.tile_pool(name="aqT", bufs=10) as aqT_p,
            tc.tile_pool(name="ysb", bufs=6) as y_p,
            tc.tile_pool(name="psum", bufs=4, space="PSUM") as ps_p,
        ):
            wscb = wqT_p.tile([P, 1], F32, tag="wscb")
            wqT = wqT_p.tile([P, KB, O], BF16, tag="wqT")

            xgs, stats, tqs, aq2s, aqTs, psums = {}, {}, {}, {}, {}, {}

            def load_x(q):       # GpSimd SWDGE queue: x loads only, issued
                if not (0 <= q < NP):   # up front; xg recycling self-paces them
                    return
                xg = xg_p.tile([P, 2 * D], BF16, tag="xg")
                if q == 0:       # two single-tile dmas: reduce(0) starts sooner
                    nc.gpsimd.dma_start(out=xg[:, 0:D], in_=xa3[:, 0, :])
                    nc.gpsimd.dma_start(out=xg[:, D:2 * D], in_=xa3[:, 1, :])
                else:
                    nc.gpsimd.dma_start(out=xg[:], in_=xa3[:, 2 * q:2 * q + 2, :])
                xgs[q] = xg

            def reduce_pair(q):  # DVE: a = absmax(x_row), both tiles in one op
                if not (0 <= q < NP):
                    return
                stats[q] = (
                    stat_p.tile([P, 2], F32, tag="st", name=f"st{q}"),
                    stat_p.tile([P, 2], F32, tag="rec", name=f"rec{q}"),
                    stat_p.tile([P, 2], F32, tag="epi", name=f"epi{q}"),
                )
                st, _, _ = stats[q]
                nc.vector.tensor_reduce(
                    st[:], xgs[q][:].rearrange("p (a d) -> p a d", d=D),
                    mybir.AxisListType.X, ALU.max, apply_absolute_value=True,
                )

            def stats_pair(q):   # DVE: rec127 = 127/a ; Scalar: epi = a*ws/127
                if not (0 <= q < NP):
                    return
                st, rec, epi = stats[q]
                nc.vector.reciprocal(rec[:], st[:])
                nc.vector.tensor_scalar(rec[:], rec[:], 127.0, None, ALU.mult)
                nc.scalar.activation(
                    epi[:], st[:], AF.Copy, bias=0.0, scale=ws127_b
                )

            def quant(t):        # DVE: aq = round-half-even(x*rec127) -> bf16
                if not (0 <= t < NT):
                    return
                q, j = t // 2, t % 2
                _, rec, _ = stats[q]
                tq = tq_p.tile([P, D], F32, tag="tq")
                nc.vector.tensor_scalar(
                    tq[:], xgs[q][:, j * D:(j + 1) * D], rec[:, j:j + 1], RND,
                    ALU.mult, ALU.add,
                )
                if j == 0:
                    aq2s[q] = aq2_p.tile(
                        [P, 2 * D], BF16, tag="aq2", name=f"aq2_{q}"
                    )
                nc.vector.tensor_scalar(
                    aq2s[q][:, j * D:(j + 1) * D], tq[:], RND, None, ALU.subtract
                )
                if j == 1:
                    del xgs[q]

            def transpose(q):    # Sync HWDGE: [s, i] -> [i, s] for both tiles
                if not (0 <= q < NP):
                    return
                aqT = aqT_p.tile([P, 2 * KB, P], BF16, tag="aqT")
                nc.sync.dma_start_transpose(aqT[:], aq2s.pop(q)[:])
                aqTs[q] = aqT

            def matmuls(q):      # PE: 32 MMs per pair (2 tiles x 2 o-banks x 8)
                if not (0 <= q < NP):
                    return
                aqT = aqTs.pop(q)
                for j in range(2):
                    yt = ps_p.tile([P, O], F32)
                    for bank in range(2):
                        o0 = bank * 512
                        for b2 in range(KB):
                            nc.tensor.matmul(
                                yt[:, o0:o0 + 512], aqT[:, j * KB + b2, :],
                                wqT[:, b2, o0:o0 + 512],
                                start=(b2 == 0), stop=(b2 == KB - 1),
                            )
                    psums[2 * q + j] = yt

            def epilogue_t(t):   # Scalar: y = psum * epi -> bf16, then store;
                if not (0 <= t < NT):   # per-tile so it overlaps the pair's
                    return              # second matmul block
                q, j = t // 2, t % 2
                _, _, epi = stats[q]
                ysb = y_p.tile([P, O], BF16, tag="ysb")
                nc.scalar.activation(
                    ysb[:], psums.pop(t)[:], AF.Copy,
                    bias=0.0, scale=epi[:, j:j + 1],
                )
                nc.scalar.dma_start(out=ya3[:, t, :], in_=ysb[:])
                if j == 1:
                    del stats[q]

            # prologue: x pairs 0-5 + weights in flight; pair 0 quantized and
            # transposed, pairs 1-2 stats done, before the steady loop
            load_x(0)
            nc.scalar.dma_start(out=wscb[:], in_=sca[:, :])
            ws127_b = wscb[:, 0:1]
            # weight halves ride the x queue BEHIND x pair 0 so the pair-0
            # chain owns the DMA engines for its first ~3us
            nc.gpsimd.dma_start(out=wqT[:, :, 0:512], in_=wa3[:, :, 0:512])
            load_x(1)
            load_x(2)
            load_x(3)
            nc.gpsimd.dma_start(out=wqT[:, :, 512:1024], in_=wa3[:, :, 512:1024])
            for q in range(4, NP):
                load_x(q)
            reduce_pair(0)
            stats_pair(0)
            quant(0)
            quant(1)
            transpose(0)
            reduce_pair(1)
            stats_pair(1)
            reduce_pair(2)
            stats_pair(2)

            # steady state: one s-tile pair per slot.  Stats run two pairs
            # ahead of the quants; the DVE block ends with the quants so the
            # transpose's counting-sem threshold lands exactly on them.
            for q in range(1, NP + 1):
                reduce_pair(q + 2)
                stats_pair(q + 2)
                quant(2 * q)
                quant(2 * q + 1)
                transpose(q)
                matmuls(q - 1)
                epilogue_t(2 * (q - 1))
                epilogue_t(2 * (q - 1) + 1)
    nc.compile()
    return nc


def _wq_host(weight):
    # mirror of reference's f32 math: ws = mean|w| + eps in f32; ternary via
    # round-half-even(w * (1/ws)) clipped to [-1, 1] (f32, like the device
    # RND-trick chain this replaces).
    m = np.abs(weight.astype(np.float64)).mean()
    ws = np.float32(np.float32(m) + np.float32(EPS))
    recw = np.float32(1.0 / np.float64(ws))
    u = (weight.astype(np.float32) * recw).astype(np.float32)
    wq = np.clip(np.round(u), -1.0, 1.0).astype(np.float32)
    ws127 = np.float32(np.float64(ws) / 127.0)
    return wq, ws127


def kernel(x, weight):
    import ml_dtypes

    weight = np.ascontiguousarray(np.asarray(weight), dtype=np.float32)
    x = np.asarray(x)
    assert x.shape == (B, S, D) and weight.shape == (O, D)
    # x in bf16: halves the dominant HBM read and doubles the DVE reduce
    # rate.  a_scale and the int8 rounding shift by <=0.4%, flipping ~8% of
    # quantized ints by +-1 -> y error std ~7 int units vs the ~125 budget.
    x = np.ascontiguousarray(x.astype(np.float32).astype(ml_dtypes.bfloat16))
    nc = _CACHE.get("nc")
    if nc is None:
        nc = _CACHE["nc"] = _build()
    wq, ws127 = _wq_host(weight)
    wqT = np.ascontiguousarray(wq.T).astype(ml_dtypes.bfloat16)
    wsc = np.full((P, 1), ws127, dtype=np.float32)
    in_maps = [{"x": x[c], "wqT": wqT, "wsc": wsc} for c in range(B)]
    trace = bool(int(os.environ.get("BITLINEAR_TRACE", "0")))
    res = run_bass_kernel_spmd(
        nc, in_maps, list(range(B)), trace=trace, tmpdir=TRACE_DIR
    )
    _CACHE["last"] = res
    return np.stack(
        [res.results[c]["y"].astype(np.float32) for c in range(B)], axis=0
    )
